# revision 9
# baseline (speedup 1.0000x reference)
"""MultiHeadAttention Trainium2 kernel (8 NeuronCores, SPMD).

Sharding: core c -> batch b = c//4, head group g = c%4 (4 heads of D=64).
Device works in transposed ("feature-major") space so no on-device
transposes are needed:
  QT/KT [256, 2048] = (x @ W.T).T per head-group, V [2048k, 4*65] with a
  ones column per head (PV matmul row 64 = softmax denominator s).
  scores_T[k, q] = KT_h.T @ QT_h (2 heads row-packed in the PE array),
  exp on ScalarE (PSUM -> SBUF, bf16), PV accumulates over k-blocks.
Host: normalizes + head-averages the shipped exp tiles (avg_attn) and
sums the 4 out-projection partials per batch.
"""

import numpy as np
import ml_dtypes

from contextlib import ExitStack

import concourse.bass as bass
import concourse.bacc as bacc
import concourse.tile as tile
from concourse import mybir
from concourse.bass_utils import run_bass_kernel_spmd

BF16 = mybir.dt.bfloat16
F32 = mybir.dt.float32
NPBF16 = ml_dtypes.bfloat16

B, S, E, H = 2, 2048, 1024, 16
D = E // H            # 64
NCORES = 8
GH = 4                # heads per core
GF = GH * D           # 256 features per core
SCALE = D ** -0.5     # 1/8

NKB = S // 128        # 16 k-blocks
NQB = S // 512        # 4 q-blocks
NEB = E // 128        # 8 e-blocks (contraction tiles for projections)

LAST_EXEC_NS = None
LAST_RESULTS = None
TRACE = False


def _ensure_ntff_hook():
    """This image's antenv lacks axon_hooks; provide it + register the
    ctypes NTFF hook so run_bass_kernel_spmd(trace=True) can profile."""
    import sys, types
    try:
        import antenv.axon_hooks  # noqa: F401
        return True
    except ImportError:
        pass
    try:
        import antenv
        from trn_agent_boot.trn_boot import _ntff_profile_via_ctypes
        hook = _ntff_profile_via_ctypes("/opt/axon/libaxon_pjrt.so")
        mod = types.ModuleType("antenv.axon_hooks")
        mod._hook = hook

        def set_axon_ntff_profile_hook(h):
            mod._hook = h

        def get_axon_ntff_profile_hook():
            return mod._hook

        mod.set_axon_ntff_profile_hook = set_axon_ntff_profile_hook
        mod.get_axon_ntff_profile_hook = get_axon_ntff_profile_hook
        sys.modules["antenv.axon_hooks"] = mod
        antenv.axon_hooks = mod
        return hook is not None
    except Exception:
        return False


def _build_module():
    nc = bacc.Bacc("TRN2", target_bir_lowering=False, debug=False,
                   num_devices=NCORES)

    # ---- I/O ----
    xqT = nc.dram_tensor("xqT", [E, S], BF16, kind="ExternalInput").ap()
    xkT = nc.dram_tensor("xkT", [E, S], BF16, kind="ExternalInput").ap()
    xvT = nc.dram_tensor("xvT", [E, S], BF16, kind="ExternalInput").ap()
    wqT = nc.dram_tensor("wqT", [E, GF], BF16, kind="ExternalInput").ap()
    wkT = nc.dram_tensor("wkT", [E, GF], BF16, kind="ExternalInput").ap()
    wvT = nc.dram_tensor("wvT", [E, GF], BF16, kind="ExternalInput").ap()
    woT = nc.dram_tensor("woT", [GF, E], BF16, kind="ExternalInput").ap()
    bqv = nc.dram_tensor("bqv", [GF, 1], F32, kind="ExternalInput").ap()
    bkv = nc.dram_tensor("bkv", [GF, 1], F32, kind="ExternalInput").ap()

    # exp_out[hl, qb, p, kb, qi]: head-local hl, q-block qb, partition p
    # (k = kb*128 + p, q = qb*512 + qi)
    exp_out = nc.dram_tensor("exp_out", [GH, NQB, 128, NKB, 512], BF16,
                             kind="ExternalOutput").ap()
    s_out = nc.dram_tensor("s_out", [GH, S], F32, kind="ExternalOutput").ap()
    out_part = nc.dram_tensor("out_part", [E, S], F32,
                              kind="ExternalOutput").ap()


    with tile.TileContext(nc) as tc, ExitStack() as ctx:
        consts = ctx.enter_context(tc.tile_pool(name="consts", bufs=1))

        # persistent activations
        persist = ctx.enter_context(tc.tile_pool(name="persist", bufs=1))
        qt0 = persist.tile([128, S], BF16, name="qt0")
        qt1 = persist.tile([128, S], BF16, name="qt1")
        kt0 = persist.tile([128, S], BF16, name="kt0")
        kt1 = persist.tile([128, S], BF16, name="kt1")
        QT = [qt0, qt1]
        KT = [kt0, kt1]
        # V: [128, kb, hl, 65]; col 64 of each head's 65 = ones
        v_sb = persist.tile([128, NKB, GH, D + 1], BF16, name="v_sb")
        attT0 = persist.tile([128, S], BF16, name="attT0")
        attT1 = persist.tile([128, S], BF16, name="attT1")
        ATT = [attT0, attT1]

        ones_sb = consts.tile([1, D], F32, name="ones_sb")
        nc.vector.memset(ones_sb[:], 1.0)
        nc.vector.memset(v_sb[:], 1.0)  # ones cols; rest overwritten

        # biases -> [128, 1] per pair-tile
        bq_sb = consts.tile([128, 2], F32, name="bq_sb")
        bk_sb = consts.tile([128, 2], F32, name="bk_sb")
        for pp in range(2):
            nc.sync.dma_start(out=bq_sb[:, pp:pp + 1],
                              in_=bqv[pp * 128:(pp + 1) * 128, :])
            nc.sync.dma_start(out=bk_sb[:, pp:pp + 1],
                              in_=bkv[pp * 128:(pp + 1) * 128, :])

        # weights resident: [128, e, fanout]
        wq_sb = consts.tile([128, NEB, GF], BF16, name="wq_sb")
        wk_sb = consts.tile([128, NEB, GF], BF16, name="wk_sb")
        wv_sb = consts.tile([128, NEB, GF], BF16, name="wv_sb")
        for e in range(NEB):
            nc.sync.dma_start(out=wq_sb[:, e, :],
                              in_=wqT[e * 128:(e + 1) * 128, :])
            nc.sync.dma_start(out=wk_sb[:, e, :],
                              in_=wkT[e * 128:(e + 1) * 128, :])
            nc.sync.dma_start(out=wv_sb[:, e, :],
                              in_=wvT[e * 128:(e + 1) * 128, :])
        wo_sb0 = consts.tile([128, E], BF16, name="wo_sb0")
        wo_sb1 = consts.tile([128, E], BF16, name="wo_sb1")
        WO = [wo_sb0, wo_sb1]
        nc.sync.dma_start(out=wo_sb0[:], in_=woT[0:128, :])
        nc.sync.dma_start(out=wo_sb1[:], in_=woT[128:256, :])

        # ---------------- projections ----------------
        with tc.tile_pool(name="xpool", bufs=10) as xpool, \
             tc.tile_pool(name="proj_ps", bufs=4, space="PSUM") as proj_ps:

            def qk_proj(xdram, w_sb, b_sb, dst_pair):
                xt = []
                for e in range(NEB):
                    x_e = xpool.tile([128, S], BF16, tag="x", name=f"x_{e}")
                    nc.sync.dma_start(out=x_e[:],
                                      in_=xdram[e * 128:(e + 1) * 128, :])
                    xt.append(x_e)
                for pp in range(2):
                    for nt in range(4):
                        ps = proj_ps.tile([128, 512], F32, tag="pp",
                                          name="ps_qk")
                        for e in range(NEB):
                            nc.tensor.matmul(
                                ps[:],
                                lhsT=w_sb[:, e, pp * 128:(pp + 1) * 128],
                                rhs=xt[e][:, nt * 512:(nt + 1) * 512],
                                start=(e == 0), stop=(e == NEB - 1))
                        nc.vector.tensor_scalar_add(
                            out=dst_pair[pp][:, nt * 512:(nt + 1) * 512],
                            in0=ps[:], scalar1=b_sb[:, pp:pp + 1])

            qk_proj(xqT, wq_sb, bq_sb, QT)
            qk_proj(xkT, wk_sb, bk_sb, KT)

            # V projection: out[kb*128+p, feat]; lhsT = xvT block (stationary)
            xvt = []
            for e in range(NEB):
                xv_e = xpool.tile([128, S], BF16, tag="x", name=f"xv_{e}")
                nc.sync.dma_start(out=xv_e[:],
                                  in_=xvT[e * 128:(e + 1) * 128, :])
                xvt.append(xv_e)
            for kb in range(NKB):
                ps = proj_ps.tile([128, GF], F32, tag="pp", name="ps_v")
                for e in range(NEB):
                    nc.tensor.matmul(
                        ps[:],
                        lhsT=xvt[e][:, kb * 128:(kb + 1) * 128],
                        rhs=wv_sb[:, e, :],
                        start=(e == 0), stop=(e == NEB - 1))
                # strided copy into [128, hl, 0:64]
                nc.vector.tensor_copy(
                    out=v_sb[:, kb, :, 0:D],
                    in_=ps.rearrange("p (h d) -> p h d", h=GH))

        # ---------------- attention ----------------
        with tc.tile_pool(name="sc_ps", bufs=2, space="PSUM") as sc_ps, \
             tc.tile_pool(name="pv_ps", bufs=2, space="PSUM") as pv_ps, \
             tc.tile_pool(name="rb_ps", bufs=2, space="PSUM") as rb_ps, \
             tc.tile_pool(name="expp", bufs=2) as expp, \
             tc.tile_pool(name="small", bufs=4) as small:

            for p in range(2):
                for qb in range(NQB):
                    qsl = slice(qb * 512, (qb + 1) * 512)
                    pva = pv_ps.tile([128, 512], F32, tag="pv", name="pva")
                    pvb = pv_ps.tile([128, 512], F32, tag="pv", name="pvb")
                    exp_t = expp.tile([128, NKB, 1024], BF16, tag="exp",
                                      name="exp_t")
                    for kb in range(NKB):
                        ksl = slice(kb * 128, (kb + 1) * 128)
                        sc = sc_ps.tile([128, 1024], F32, tag="sc", name="sc")
                        # two heads row-packed (K=64 each)
                        nc.tensor.matmul(sc[:, 0:512],
                                         lhsT=KT[p][0:64, ksl],
                                         rhs=QT[p][0:64, qsl],
                                         start=True, stop=True)
                        nc.tensor.matmul(sc[:, 512:1024],
                                         lhsT=KT[p][64:128, ksl],
                                         rhs=QT[p][64:128, qsl],
                                         start=True, stop=True)
                        nc.scalar.activation(
                            out=exp_t[:, kb, :], in_=sc[:],
                            func=mybir.ActivationFunctionType.Exp)
                        nc.tensor.matmul(pva[0:D + 1, :],
                                         lhsT=v_sb[:, kb, 2 * p, :],
                                         rhs=exp_t[:, kb, 0:512],
                                         start=(kb == 0), stop=(kb == NKB - 1))
                        nc.tensor.matmul(pvb[0:D + 1, :],
                                         lhsT=v_sb[:, kb, 2 * p + 1, :],
                                         rhs=exp_t[:, kb, 512:1024],
                                         start=(kb == 0), stop=(kb == NKB - 1))

                    # denominators, broadcast r, normalize attended
                    rbc_sbs = []
                    for hi, pv in ((0, pva), (1, pvb)):
                        hl = 2 * p + hi
                        s_sb = small.tile([1, 512], F32, tag="s", name="s_sb")
                        nc.vector.tensor_copy(out=s_sb[:], in_=pv[D:D + 1, :])
                        nc.sync.dma_start(
                            out=s_out[hl:hl + 1, qsl], in_=s_sb[:])
                        r_sb = small.tile([1, 512], F32, tag="r", name="r_sb")
                        nc.vector.reciprocal(out=r_sb[:], in_=s_sb[:])
                        rbc = rb_ps.tile([D, 512], F32, tag="rb", name="rbc")
                        nc.tensor.matmul(rbc[:], lhsT=ones_sb[:], rhs=r_sb[:],
                                         start=True, stop=True)
                        rbc_sb = small.tile([D, 512], F32, tag="rbs",
                                            name="rbc_sb", bufs=4)
                        nc.vector.tensor_copy(out=rbc_sb[:], in_=rbc[:])
                        rbc_sbs.append(rbc_sb)
                    # head A: partitions 0:64 align
                    nc.vector.tensor_mul(out=ATT[p][0:D, qsl],
                                         in0=pva[0:D, :],
                                         in1=rbc_sbs[0][:])
                    # head B: compute at partitions 0:64, then SBUF->SBUF DMA
                    # shift to partitions 64:128
                    att_b = small.tile([D, 512], BF16, tag="attb",
                                       name="att_b")
                    nc.vector.tensor_mul(out=att_b[:], in0=pvb[0:D, :],
                                         in1=rbc_sbs[1][:])
                    nc.sync.dma_start(out=ATT[p][D:2 * D, qsl], in_=att_b[:])

                    # ship exp tiles (unnormalized; host divides by s)
                    nc.sync.dma_start(out=exp_out[2 * p, qb],
                                      in_=exp_t[:, :, 0:512])
                    nc.sync.dma_start(out=exp_out[2 * p + 1, qb],
                                      in_=exp_t[:, :, 512:1024])

        # ---------------- output projection ----------------
        with tc.tile_pool(name="op_ps", bufs=4, space="PSUM") as op_ps, \
             tc.tile_pool(name="op_sb", bufs=4) as op_sb:
            for mt in range(8):
                for nt in range(4):
                    ps = op_ps.tile([128, 512], F32, tag="op", name="ps_o")
                    for kt in range(2):
                        nc.tensor.matmul(
                            ps[:],
                            lhsT=WO[kt][:, mt * 128:(mt + 1) * 128],
                            rhs=ATT[kt][:, nt * 512:(nt + 1) * 512],
                            start=(kt == 0), stop=(kt == 1))
                    o_sb = op_sb.tile([128, 512], F32, tag="ob", name="o_sb")
                    nc.vector.tensor_copy(out=o_sb[:], in_=ps[:])
                    nc.sync.dma_start(
                        out=out_part[mt * 128:(mt + 1) * 128,
                                     nt * 512:(nt + 1) * 512],
                        in_=o_sb[:])

    nc.compile()
    return nc


_NC_CACHE = None


def _get_module():
    global _NC_CACHE
    if _NC_CACHE is None:
        _NC_CACHE = _build_module()
    return _NC_CACHE


def kernel(query, key, value, Wq, bq, Wk, bk, Wv, bv, Wo, bo):
    global LAST_EXEC_NS, LAST_RESULTS
    query = np.asarray(query, np.float32)
    key = np.asarray(key, np.float32)
    value = np.asarray(value, np.float32)
    Wq = np.asarray(Wq, np.float32)
    Wk = np.asarray(Wk, np.float32)
    Wv = np.asarray(Wv, np.float32)
    Wo = np.asarray(Wo, np.float32)
    bq = np.asarray(bq, np.float32)
    bk = np.asarray(bk, np.float32)
    bv = np.asarray(bv, np.float32)
    bo = np.asarray(bo, np.float32)

    nc = _get_module()

    in_maps = []
    for c in range(NCORES):
        b, g = divmod(c, 4)
        cols = slice(g * GF, (g + 1) * GF)
        in_maps.append({
            "xqT": np.ascontiguousarray(query[b].T).astype(NPBF16),
            "xkT": np.ascontiguousarray(key[b].T).astype(NPBF16),
            "xvT": np.ascontiguousarray(value[b].T).astype(NPBF16),
            # scores scale folded into Wq/bq
            "wqT": np.ascontiguousarray((Wq[cols, :] * SCALE).T).astype(NPBF16),
            "wkT": np.ascontiguousarray(Wk[cols, :].T).astype(NPBF16),
            "wvT": np.ascontiguousarray(Wv[cols, :].T).astype(NPBF16),
            "woT": np.ascontiguousarray(Wo[:, cols].T).astype(NPBF16),
            "bqv": (bq[cols] * SCALE).astype(np.float32).reshape(GF, 1),
            "bkv": bk[cols].astype(np.float32).reshape(GF, 1),
        })

    trace = TRACE and _ensure_ntff_hook()
    res = run_bass_kernel_spmd(nc, in_maps, core_ids=list(range(NCORES)),
                               trace=trace)
    LAST_EXEC_NS = res.exec_time_ns
    LAST_RESULTS = res

    # ---- host combine ----
    output = np.zeros((B, S, E), np.float32)
    avg = np.zeros((B, S, S), np.float32)  # [b, k, q] for now
    for c in range(NCORES):
        b, g = divmod(c, 4)
        r = res.results[c]
        output[b] += r["out_part"].T
        ex = r["exp_out"].astype(np.float32)   # [hl, qb, p, kb, qi]
        s = r["s_out"].astype(np.float32)      # [hl, q]
        rn = (1.0 / s).reshape(GH, NQB, 1, 1, 512)
        contrib = (ex * rn).sum(axis=0)        # [qb, p, kb, qi]
        # k = kb*128 + p, q = qb*512 + qi
        avg[b] += contrib.transpose(2, 1, 0, 3).reshape(S, S)
    avg = (avg / H).transpose(0, 2, 1)         # -> [b, q, k]
    avg = np.ascontiguousarray(avg)

    # bv folded through out-proj + bo
    output += (bv @ Wo.T + bo)[None, None, :]
    return output, avg


# revision 15
# speedup vs baseline: 1.2230x; 1.2230x over previous
"""MultiHeadAttention Trainium2 kernel (8 NeuronCores, SPMD).

Sharding: core c -> batch b = c//4, head group g = c%4 (4 heads of D=64).
Device works in transposed ("feature-major") space so no on-device
transposes are needed:
  QT/KT [256, 2048] = (x @ W.T).T per head-group, V [2048k, 4*65] with a
  ones column per head (PV matmul row 64 = softmax denominator s).
  scores_T[k, q] = KT_h.T @ QT_h (2 heads row-packed in the PE array),
  exp on ScalarE (PSUM -> SBUF, bf16), PV accumulates over k-blocks.
Host: normalizes + head-averages the shipped exp tiles (avg_attn) and
sums the 4 out-projection partials per batch.
"""

import numpy as np
import ml_dtypes

from contextlib import ExitStack

import concourse.bass as bass
import concourse.bacc as bacc
import concourse.tile as tile
from concourse import mybir
from concourse.bass_utils import run_bass_kernel_spmd

BF16 = mybir.dt.bfloat16
F32 = mybir.dt.float32
NPBF16 = ml_dtypes.bfloat16

B, S, E, H = 2, 2048, 1024, 16
D = E // H            # 64
NCORES = 8
GH = 4                # heads per core
GF = GH * D           # 256 features per core
SCALE = D ** -0.5     # 1/8

NKB = S // 128        # 16 k-blocks
NQB = S // 512        # 4 q-blocks
NEB = E // 128        # 8 e-blocks (contraction tiles for projections)

LAST_EXEC_NS = None
LAST_RESULTS = None
TRACE = False


def _ensure_ntff_hook():
    """This image's antenv lacks axon_hooks; provide it + register the
    ctypes NTFF hook so run_bass_kernel_spmd(trace=True) can profile."""
    import sys, types
    try:
        import antenv.axon_hooks  # noqa: F401
        return True
    except ImportError:
        pass
    try:
        import antenv
        from trn_agent_boot.trn_boot import _ntff_profile_via_ctypes
        hook = _ntff_profile_via_ctypes("/opt/axon/libaxon_pjrt.so")
        mod = types.ModuleType("antenv.axon_hooks")
        mod._hook = hook

        def set_axon_ntff_profile_hook(h):
            mod._hook = h

        def get_axon_ntff_profile_hook():
            return mod._hook

        mod.set_axon_ntff_profile_hook = set_axon_ntff_profile_hook
        mod.get_axon_ntff_profile_hook = get_axon_ntff_profile_hook
        sys.modules["antenv.axon_hooks"] = mod
        antenv.axon_hooks = mod
        return hook is not None
    except Exception:
        return False


def _build_module():
    nc = bacc.Bacc("TRN2", target_bir_lowering=False, debug=False,
                   num_devices=NCORES)

    # ---- I/O ----
    xqT = nc.dram_tensor("xqT", [E, S], BF16, kind="ExternalInput").ap()
    xkT = nc.dram_tensor("xkT", [E, S], BF16, kind="ExternalInput").ap()
    xvT = nc.dram_tensor("xvT", [E, S], BF16, kind="ExternalInput").ap()
    wqT = nc.dram_tensor("wqT", [E, GF], BF16, kind="ExternalInput").ap()
    wkT = nc.dram_tensor("wkT", [E, GF], BF16, kind="ExternalInput").ap()
    wvT = nc.dram_tensor("wvT", [E, GF], BF16, kind="ExternalInput").ap()
    woT = nc.dram_tensor("woT", [GF, E], BF16, kind="ExternalInput").ap()
    bqv = nc.dram_tensor("bqv", [GF, 1], F32, kind="ExternalInput").ap()
    bkv = nc.dram_tensor("bkv", [GF, 1], F32, kind="ExternalInput").ap()

    # exp_out[hl, qb, p, kb, qi]: head-local hl, q-block qb, partition p
    # (k = kb*128 + p, q = qb*512 + qi)
    exp_out = nc.dram_tensor("exp_out", [GH, NQB, 128, NKB, 512], BF16,
                             kind="ExternalOutput").ap()
    s_out = nc.dram_tensor("s_out", [GH, S], F32, kind="ExternalOutput").ap()
    out_part = nc.dram_tensor("out_part", [E, S], F32,
                              kind="ExternalOutput").ap()


    with tile.TileContext(nc) as tc, ExitStack() as ctx:
        consts = ctx.enter_context(tc.tile_pool(name="consts", bufs=1))

        # persistent activations
        persist = ctx.enter_context(tc.tile_pool(name="persist", bufs=1))
        qt0 = persist.tile([128, S], BF16, name="qt0")
        qt1 = persist.tile([128, S], BF16, name="qt1")
        kt0 = persist.tile([128, S], BF16, name="kt0")
        kt1 = persist.tile([128, S], BF16, name="kt1")
        QT = [qt0, qt1]
        KT = [kt0, kt1]
        # V: [128, kb, hl, 65]; col 64 of each head's 65 = ones
        v_sb = persist.tile([128, NKB, GH, D + 1], BF16, name="v_sb")
        attT0 = persist.tile([128, S], BF16, name="attT0")
        attT1 = persist.tile([128, S], BF16, name="attT1")
        ATT = [attT0, attT1]

        nc.vector.memset(v_sb[:], 1.0)  # ones cols; rest overwritten

        # biases -> [128, 1] per pair-tile
        bq_sb = consts.tile([128, 2], F32, name="bq_sb")
        bk_sb = consts.tile([128, 2], F32, name="bk_sb")
        for pp in range(2):
            nc.sync.dma_start(out=bq_sb[:, pp:pp + 1],
                              in_=bqv[pp * 128:(pp + 1) * 128, :])
            nc.sync.dma_start(out=bk_sb[:, pp:pp + 1],
                              in_=bkv[pp * 128:(pp + 1) * 128, :])

        # weights resident: [128, e, fanout]
        wq_sb = consts.tile([128, NEB, GF], BF16, name="wq_sb")
        wk_sb = consts.tile([128, NEB, GF], BF16, name="wk_sb")
        wv_sb = consts.tile([128, NEB, GF], BF16, name="wv_sb")
        for e in range(NEB):
            nc.sync.dma_start(out=wq_sb[:, e, :],
                              in_=wqT[e * 128:(e + 1) * 128, :])
            nc.sync.dma_start(out=wk_sb[:, e, :],
                              in_=wkT[e * 128:(e + 1) * 128, :])
            nc.sync.dma_start(out=wv_sb[:, e, :],
                              in_=wvT[e * 128:(e + 1) * 128, :])
        wo_sb0 = consts.tile([128, E], BF16, name="wo_sb0")
        wo_sb1 = consts.tile([128, E], BF16, name="wo_sb1")
        WO = [wo_sb0, wo_sb1]
        nc.sync.dma_start(out=wo_sb0[:], in_=woT[0:128, :])
        nc.sync.dma_start(out=wo_sb1[:], in_=woT[128:256, :])

        # ---------------- projections ----------------
        with tc.tile_pool(name="xpool", bufs=10) as xpool, \
             tc.tile_pool(name="proj_ps", bufs=4, space="PSUM") as proj_ps:

            def qk_proj(xdram, w_sb, b_sb, dst_pair):
                xt = []
                for e in range(NEB):
                    x_e = xpool.tile([128, S], BF16, tag="x", name=f"x_{e}")
                    nc.sync.dma_start(out=x_e[:],
                                      in_=xdram[e * 128:(e + 1) * 128, :])
                    xt.append(x_e)
                # e outer / nt inner: one LDWEIGHTS feeds 4 matmuls
                for pp in range(2):
                    pss = [proj_ps.tile([128, 512], F32, tag="pp",
                                        name=f"ps_qk{nt}") for nt in range(4)]
                    for e in range(NEB):
                        for nt in range(4):
                            nc.tensor.matmul(
                                pss[nt][:],
                                lhsT=w_sb[:, e, pp * 128:(pp + 1) * 128],
                                rhs=xt[e][:, nt * 512:(nt + 1) * 512],
                                start=(e == 0), stop=(e == NEB - 1))
                    for nt in range(4):
                        nc.vector.tensor_scalar_add(
                            out=dst_pair[pp][:, nt * 512:(nt + 1) * 512],
                            in0=pss[nt][:], scalar1=b_sb[:, pp:pp + 1])

            qk_proj(xqT, wq_sb, bq_sb, QT)
            qk_proj(xkT, wk_sb, bk_sb, KT)

            # V projection: out[kb*128+p, feat]; lhsT = xvT block (stationary)
            xvt = []
            for e in range(NEB):
                xv_e = xpool.tile([128, S], BF16, tag="x", name=f"xv_{e}")
                nc.sync.dma_start(out=xv_e[:],
                                  in_=xvT[e * 128:(e + 1) * 128, :])
                xvt.append(xv_e)
            for kb in range(NKB):
                ps = proj_ps.tile([128, GF], F32, tag="pp", name="ps_v")
                for e in range(NEB):
                    nc.tensor.matmul(
                        ps[:],
                        lhsT=xvt[e][:, kb * 128:(kb + 1) * 128],
                        rhs=wv_sb[:, e, :],
                        start=(e == 0), stop=(e == NEB - 1))
                # strided copy into [128, hl, 0:64]
                nc.vector.tensor_copy(
                    out=v_sb[:, kb, :, 0:D],
                    in_=ps.rearrange("p (h d) -> p h d", h=GH))

        # ---------------- attention ----------------
        with tc.tile_pool(name="sc_ps", bufs=2, space="PSUM") as sc_ps, \
             tc.tile_pool(name="pv_ps", bufs=4, space="PSUM") as pv_ps, \
             tc.tile_pool(name="expp", bufs=3) as expp, \
             tc.tile_pool(name="small", bufs=4) as small, \
             tc.tile_pool(name="rdram", bufs=4, space="DRAM") as rdram:

            for p in range(2):
                for qb in range(NQB):
                    qsl = slice(qb * 512, (qb + 1) * 512)
                    pva = pv_ps.tile([128, 512], F32, tag="pv", name="pva")
                    pvb = pv_ps.tile([128, 512], F32, tag="pv", name="pvb")
                    exp_t = expp.tile([128, NKB, 1024], BF16, tag="exp",
                                      name="exp_t")
                    for kb in range(NKB):
                        ksl = slice(kb * 128, (kb + 1) * 128)
                        sc = sc_ps.tile([128, 1024], F32, tag="sc", name="sc")
                        # two heads row-packed (K=64 each)
                        nc.tensor.matmul(sc[:, 0:512],
                                         lhsT=KT[p][0:64, ksl],
                                         rhs=QT[p][0:64, qsl],
                                         start=True, stop=True)
                        nc.tensor.matmul(sc[:, 512:1024],
                                         lhsT=KT[p][64:128, ksl],
                                         rhs=QT[p][64:128, qsl],
                                         start=True, stop=True)
                        nc.scalar.activation(
                            out=exp_t[:, kb, :], in_=sc[:],
                            func=mybir.ActivationFunctionType.Exp)
                        nc.tensor.matmul(pva[0:D + 1, :],
                                         lhsT=v_sb[:, kb, 2 * p, :],
                                         rhs=exp_t[:, kb, 0:512],
                                         start=(kb == 0), stop=(kb == NKB - 1))
                        nc.tensor.matmul(pvb[0:D + 1, :],
                                         lhsT=v_sb[:, kb, 2 * p + 1, :],
                                         rhs=exp_t[:, kb, 512:1024],
                                         start=(kb == 0), stop=(kb == NKB - 1))

                    # denominators, broadcast r, normalize attended
                    rbc_sbs = []
                    for hi, pv in ((0, pva), (1, pvb)):
                        hl = 2 * p + hi
                        s_sb = small.tile([1, 512], F32, tag="s", name="s_sb")
                        nc.vector.tensor_copy(out=s_sb[:], in_=pv[D:D + 1, :])
                        nc.sync.dma_start(
                            out=s_out[hl:hl + 1, qsl], in_=s_sb[:])
                        r_sb = small.tile([1, 512], F32, tag="r", name="r_sb")
                        nc.vector.reciprocal(out=r_sb[:], in_=s_sb[:])
                        r_dram = rdram.tile([1, 512], F32, tag="rd",
                                            name="r_dram")
                        nc.sync.dma_start(out=r_dram[:], in_=r_sb[:])
                        rbc_sb = small.tile([D, 512], F32, tag="rbs",
                                            name="rbc_sb", bufs=4)
                        nc.gpsimd.dma_start(
                            out=rbc_sb[:],
                            in_=bass.AP(tensor=r_dram.tensor,
                                        offset=r_dram.offset,
                                        ap=[[0, D]] + list(r_dram.ap[1:])))
                        rbc_sbs.append(rbc_sb)
                    # head A: partitions 0:64 align
                    nc.vector.tensor_mul(out=ATT[p][0:D, qsl],
                                         in0=pva[0:D, :],
                                         in1=rbc_sbs[0][:])
                    # head B: compute at partitions 0:64, then SBUF->SBUF DMA
                    # shift to partitions 64:128
                    att_b = small.tile([D, 512], BF16, tag="attb",
                                       name="att_b")
                    nc.vector.tensor_mul(out=att_b[:], in0=pvb[0:D, :],
                                         in1=rbc_sbs[1][:])
                    nc.sync.dma_start(out=ATT[p][D:2 * D, qsl], in_=att_b[:])

                    # ship exp tiles (unnormalized; host divides by s)
                    nc.sync.dma_start(out=exp_out[2 * p, qb],
                                      in_=exp_t[:, :, 0:512])
                    nc.sync.dma_start(out=exp_out[2 * p + 1, qb],
                                      in_=exp_t[:, :, 512:1024])

        # ---------------- output projection ----------------
        with tc.tile_pool(name="op_ps", bufs=4, space="PSUM") as op_ps, \
             tc.tile_pool(name="op_sb", bufs=4) as op_sb:
            for mt in range(8):
                for nt in range(4):
                    ps = op_ps.tile([128, 512], F32, tag="op", name="ps_o")
                    for kt in range(2):
                        nc.tensor.matmul(
                            ps[:],
                            lhsT=WO[kt][:, mt * 128:(mt + 1) * 128],
                            rhs=ATT[kt][:, nt * 512:(nt + 1) * 512],
                            start=(kt == 0), stop=(kt == 1))
                    o_sb = op_sb.tile([128, 512], F32, tag="ob", name="o_sb")
                    nc.vector.tensor_copy(out=o_sb[:], in_=ps[:])
                    nc.sync.dma_start(
                        out=out_part[mt * 128:(mt + 1) * 128,
                                     nt * 512:(nt + 1) * 512],
                        in_=o_sb[:])

    nc.compile()
    return nc


_NC_CACHE = None


def _get_module():
    global _NC_CACHE
    if _NC_CACHE is None:
        _NC_CACHE = _build_module()
    return _NC_CACHE


def kernel(query, key, value, Wq, bq, Wk, bk, Wv, bv, Wo, bo):
    global LAST_EXEC_NS, LAST_RESULTS
    query = np.asarray(query, np.float32)
    key = np.asarray(key, np.float32)
    value = np.asarray(value, np.float32)
    Wq = np.asarray(Wq, np.float32)
    Wk = np.asarray(Wk, np.float32)
    Wv = np.asarray(Wv, np.float32)
    Wo = np.asarray(Wo, np.float32)
    bq = np.asarray(bq, np.float32)
    bk = np.asarray(bk, np.float32)
    bv = np.asarray(bv, np.float32)
    bo = np.asarray(bo, np.float32)

    nc = _get_module()

    in_maps = []
    for c in range(NCORES):
        b, g = divmod(c, 4)
        cols = slice(g * GF, (g + 1) * GF)
        in_maps.append({
            "xqT": np.ascontiguousarray(query[b].T).astype(NPBF16),
            "xkT": np.ascontiguousarray(key[b].T).astype(NPBF16),
            "xvT": np.ascontiguousarray(value[b].T).astype(NPBF16),
            # scores scale folded into Wq/bq
            "wqT": np.ascontiguousarray((Wq[cols, :] * SCALE).T).astype(NPBF16),
            "wkT": np.ascontiguousarray(Wk[cols, :].T).astype(NPBF16),
            "wvT": np.ascontiguousarray(Wv[cols, :].T).astype(NPBF16),
            "woT": np.ascontiguousarray(Wo[:, cols].T).astype(NPBF16),
            "bqv": (bq[cols] * SCALE).astype(np.float32).reshape(GF, 1),
            "bkv": bk[cols].astype(np.float32).reshape(GF, 1),
        })

    trace = TRACE and _ensure_ntff_hook()
    res = run_bass_kernel_spmd(nc, in_maps, core_ids=list(range(NCORES)),
                               trace=trace)
    LAST_EXEC_NS = res.exec_time_ns
    LAST_RESULTS = res

    # ---- host combine ----
    output = np.zeros((B, S, E), np.float32)
    avg = np.zeros((B, S, S), np.float32)  # [b, k, q] for now
    for c in range(NCORES):
        b, g = divmod(c, 4)
        r = res.results[c]
        output[b] += r["out_part"].T
        ex = r["exp_out"].astype(np.float32)   # [hl, qb, p, kb, qi]
        s = r["s_out"].astype(np.float32)      # [hl, q]
        rn = (1.0 / s).reshape(GH, NQB, 1, 1, 512)
        contrib = (ex * rn).sum(axis=0)        # [qb, p, kb, qi]
        # k = kb*128 + p, q = qb*512 + qi
        avg[b] += contrib.transpose(2, 1, 0, 3).reshape(S, S)
    avg = (avg / H).transpose(0, 2, 1)         # -> [b, q, k]
    avg = np.ascontiguousarray(avg)

    # bv folded through out-proj + bo
    output += (bv @ Wo.T + bo)[None, None, :]
    return output, avg
